# revision 3
# baseline (speedup 1.0000x reference)
"""Trainium2 Bass kernel for BioNet message-passing recurrence.

Computes 50 steps of  X <- mml(W @ X + X_bias)  with W (8192x8192 f32,
masked) and X (8192x32), returning X.T (32, 8192).

Strategy (8 NeuronCores, tensor-parallel over W rows):
  - Each core holds rows [1024c, 1024c+1024) of W, stored transposed in
    SBUF as bf16 (16.8 MB/core) for the whole kernel -> no per-step HBM
    traffic for W.
  - Per step, each core computes its 1024 rows of W @ X as
    out^T = X^T @ W_shard^T on the PE with X (128,32) tiles stationary
    and W streaming, 4-way column-tiled (4 concurrent 32-wide stationary
    tiles, one per K-subset) for ~4x PE throughput at batch=32.
  - The 4 column-group partials land on partition groups 32j..32j+32 of
    PSUM; a second small PE pass multiplies by a selector matrix
    S[p,b] = (p%32==b) which fuses the 4-way reduction with the
    (batch,node) -> (node,batch) transpose.
  - Bias + Michaelis-Menten activation on DVE, then the (1024,32) bf16
    chunk is AllGathered across the 8 cores for the next step.
"""

import os
import sys
import types

sys.path.insert(0, "/opt/trn_rl_repo")

import numpy as np
import ml_dtypes

import concourse.bass as bass
import concourse.mybir as mybir
import concourse.tile as tile
from concourse import bacc
import concourse.bass_utils as bass_utils
from concourse.bass import ts
from concourse.bass_utils import run_bass_kernel_spmd

N_NODES = 8192
N_CORES = 8
BATCH = 32
MAX_STEPS = 50
LEAK = 0.01
LOCAL = N_NODES // N_CORES          # 1024 rows per core
K_TILES = N_NODES // 128            # 64
LOCAL_TILES = LOCAL // 128          # 8
CHUNK_F = LOCAL_TILES * BATCH       # 256 free elems per activated chunk

LAST_RESULTS = None  # BassKernelResults of the most recent run (for test.py)


def setup_tracing():
    """Register the axon NTFF profile hook; the container's antenv is a stub."""
    try:
        import antenv
        if "antenv.axon_hooks" not in sys.modules:
            mod = types.ModuleType("antenv.axon_hooks")
            mod._hook = None
            mod.set_axon_ntff_profile_hook = lambda h: setattr(mod, "_hook", h)
            mod.get_axon_ntff_profile_hook = lambda: mod._hook
            sys.modules["antenv.axon_hooks"] = mod
            antenv.axon_hooks = mod
            from trn_agent_boot.trn_boot import _ntff_profile_via_ctypes
            mod.set_axon_ntff_profile_hook(
                _ntff_profile_via_ctypes("/opt/axon/libaxon_pjrt.so")
            )
        bass_utils.upload_artifacts = lambda tmpdir: f"local://{tmpdir}"
    except Exception:
        pass


def build_nc():
    nc = bacc.Bacc(None, target_bir_lowering=False, num_devices=N_CORES)
    f32 = mybir.dt.float32
    bf16 = mybir.dt.bfloat16

    # Per-core inputs (shapes identical on every core; contents sharded).
    wt = nc.dram_tensor("wt", [N_NODES, LOCAL], bf16, kind="ExternalInput")
    xb = nc.dram_tensor("xb", [128, CHUNK_F], f32, kind="ExternalInput")
    s_in = nc.dram_tensor("s_in", [128, BATCH], f32, kind="ExternalInput")
    out = nc.dram_tensor("out", [128, CHUNK_F], f32, kind="ExternalOutput")

    with tile.TileContext(nc) as tc:
        with (
            tc.tile_pool(name="persist", bufs=1) as persist,
            tc.tile_pool(name="ys", bufs=2) as ys_pool,
            tc.tile_pool(name="chain", bufs=2) as chain,
            tc.tile_pool(name="stage", bufs=2) as stage_pool,
            tc.tile_pool(name="psum", bufs=2, space="PSUM") as psum_pool,
            tc.tile_pool(name="psumt", bufs=2, space="PSUM") as psumt_pool,
            tc.tile_pool(name="dram", bufs=2, space="DRAM") as dram,
        ):
            # ---- persistent SBUF tensors -------------------------------
            wt_sb = persist.tile([128, K_TILES, LOCAL], bf16)      # 128 KB/part
            nc.sync.dma_start(
                out=wt_sb, in_=wt.rearrange("(t p) n -> p t n", p=128)
            )
            xb_sb = persist.tile([128, CHUNK_F], f32)
            nc.sync.dma_start(out=xb_sb, in_=xb[:])
            s_sb = persist.tile([128, BATCH], f32)
            nc.sync.dma_start(out=s_sb, in_=s_in[:])
            x_sb = persist.tile([128, K_TILES * BATCH], bf16)      # gathered state

            def activation(z_src, to_bf, also_f32=None):
                """to_bf[:] = mml(z_src) in bf16; optionally also f32 copy.

                mml(z) = max(leak*z, min(z, 1 - 0.25/max(z, 0.5)))
                (exact for |z| < ~99, which holds here).
                """
                m_t = chain.tile([128, CHUNK_F], f32, tag="m")
                nc.vector.tensor_scalar_max(m_t, z_src, 0.5)
                r_t = chain.tile([128, CHUNK_F], f32, tag="r")
                nc.vector.reciprocal_approx_fast(out=r_t, in_=m_t)
                s_t = chain.tile([128, CHUNK_F], f32, tag="s")
                nc.vector.tensor_scalar(
                    s_t, r_t, -0.25, 1.0,
                    mybir.AluOpType.mult, mybir.AluOpType.add,
                )
                t_t = chain.tile([128, CHUNK_F], f32, tag="t")
                nc.vector.tensor_tensor(t_t, z_src, s_t, mybir.AluOpType.min)
                u_t = chain.tile([128, CHUNK_F], f32, tag="u")
                nc.vector.tensor_scalar_mul(u_t, z_src, LEAK)
                nc.vector.tensor_tensor(to_bf, u_t, t_t, mybir.AluOpType.max)
                if also_f32 is not None:
                    nc.vector.tensor_tensor(
                        also_f32, u_t, t_t, mybir.AluOpType.max
                    )

            def broadcast(stage_bf):
                """AllGather the local activated chunk into x_sb."""
                ag_in = dram.tile([128, CHUNK_F], bf16, tag="agi")
                nc.sync.dma_start(out=ag_in, in_=stage_bf)
                ag_out = dram.tile(
                    [128 * N_CORES, CHUNK_F], bf16, addr_space="Shared", tag="ago"
                )
                nc.gpsimd.collective_compute(
                    "AllGather",
                    mybir.AluOpType.bypass,
                    replica_groups=[list(range(N_CORES))],
                    ins=[ag_in.opt()],
                    outs=[ag_out.opt()],
                )
                nc.sync.dma_start(
                    out=x_sb.rearrange("p (c f) -> p c f", c=N_CORES),
                    in_=ag_out.rearrange("(c p) f -> p c f", p=128),
                )

            # ---- step 1: X1 = mml(X_bias) ------------------------------
            stage_bf = stage_pool.tile([128, CHUNK_F], bf16, tag="stage")
            activation(xb_sb, stage_bf)
            broadcast(stage_bf)

            # ---- steps 2..50: X <- mml(W @ X + X_bias) -----------------
            n_quads = K_TILES // 4  # 16
            for step in range(MAX_STEPS - 1):
                last = step == MAX_STEPS - 2
                # main matmul: 4-way column-tiled over K-subsets
                psum_h = [
                    psum_pool.tile(
                        [128, 512], mybir.dt.float32, tag="pa", name="psum_a"
                    ),
                    psum_pool.tile(
                        [128, 512], mybir.dt.float32, tag="pb", name="psum_b"
                    ),
                ]
                for q in range(n_quads):
                    for h in range(2):
                        for j in range(4):
                            k = 4 * q + j
                            nc.tensor.matmul(
                                psum_h[h][32 * j : 32 * (j + 1), :],
                                x_sb[:, ts(k, BATCH)],
                                wt_sb[:, k, ts(h, 512)],
                                start=(q == 0),
                                stop=(q == n_quads - 1),
                                tile_position=(0, 32 * j),
                            )
                # reduce(4 groups) + transpose via S-matrix PE pass
                ysb = ys_pool.tile([128, LOCAL], mybir.dt.float32, tag="ysb")
                psum_t = psumt_pool.tile([128, CHUNK_F], mybir.dt.float32, tag="pt")
                for t in range(LOCAL_TILES):
                    nc.vector.tensor_copy(
                        ysb[:, ts(t, 128)],
                        psum_h[t // 4][:, ts(t % 4, 128)],
                    )
                    nc.tensor.matmul(
                        psum_t[:, ts(t, BATCH)],
                        ysb[:, ts(t, 128)],
                        s_sb,
                        start=True,
                        stop=True,
                    )
                # bias + activation
                z_t = chain.tile([128, CHUNK_F], mybir.dt.float32, tag="z")
                nc.vector.tensor_tensor(z_t, psum_t, xb_sb, mybir.AluOpType.add)
                stage_bf = stage_pool.tile([128, CHUNK_F], bf16, tag="stage")
                if last:
                    out_f32 = stage_pool.tile(
                        [128, CHUNK_F], mybir.dt.float32, tag="of"
                    )
                    activation(z_t, stage_bf, also_f32=out_f32)
                    nc.sync.dma_start(out=out[:], in_=out_f32)
                else:
                    activation(z_t, stage_bf)
                    broadcast(stage_bf)

    nc.compile()
    return nc


def _prepare_in_maps(X_full, weights, bias, edge_mask):
    W = np.where(edge_mask, weights, 0.0).astype(np.float32)
    Xb = X_full.astype(np.float32).T + bias.astype(np.float32)  # (n, B)
    S = np.zeros((128, BATCH), np.float32)
    S[np.arange(128), np.arange(128) % BATCH] = 1.0
    in_maps = []
    for c in range(N_CORES):
        rows = slice(LOCAL * c, LOCAL * (c + 1))
        wt_c = np.ascontiguousarray(W[rows, :].T).astype(ml_dtypes.bfloat16)
        xb_c = (
            Xb[rows]                       # (1024, 32)
            .reshape(LOCAL_TILES, 128, BATCH)
            .transpose(1, 0, 2)
            .reshape(128, CHUNK_F)
            .copy()
        )
        in_maps.append({"wt": wt_c, "xb": xb_c, "s_in": S})
    return in_maps


def _reassemble(results):
    out = np.empty((BATCH, N_NODES), np.float32)
    for c in range(N_CORES):
        oc = np.asarray(results[c]["out"])  # (128, 256)
        chunk = (
            oc.reshape(128, LOCAL_TILES, BATCH)
            .transpose(1, 0, 2)
            .reshape(LOCAL, BATCH)
        )
        out[:, LOCAL * c : LOCAL * (c + 1)] = chunk.T
    return out


def kernel(X_full, weights, bias, edge_mask):
    global LAST_RESULTS
    setup_tracing()
    in_maps = _prepare_in_maps(X_full, weights, bias, edge_mask)
    nc = build_nc()
    res = run_bass_kernel_spmd(nc, in_maps, core_ids=list(range(N_CORES)))
    LAST_RESULTS = res
    return _reassemble(res.results)


if __name__ == "__main__":
    # quick self-run with random data
    rng = np.random.default_rng(0)
    X_full = rng.random((BATCH, N_NODES), np.float32)
    weights = rng.standard_normal((N_NODES, N_NODES), np.float32)
    bias = 0.001 * np.ones((N_NODES, 1), np.float32)
    edge_mask = rng.random((N_NODES, N_NODES)) < 0.002
    out = kernel(X_full, weights, bias, edge_mask)
    print("out", out.shape, out.dtype, out[:2, :4])


# revision 8
# speedup vs baseline: 1.1088x; 1.1088x over previous
"""Trainium2 Bass kernel for BioNet message-passing recurrence.

Computes 50 steps of  X <- mml(W @ X + X_bias)  with W (8192x8192 f32,
masked) and X (8192x32), returning X.T (32, 8192).

Strategy (8 NeuronCores, tensor-parallel over W rows):
  - Each core holds rows [1024c, 1024c+1024) of W, stored transposed in
    SBUF as bf16 (16.8 MB/core) for the whole kernel -> no per-step HBM
    traffic for W.
  - Per step, each core computes its 1024 rows of W @ X as
    out^T = X^T @ W_shard^T on the PE with X (128,32) tiles stationary
    and W streaming, 4-way column-tiled (4 concurrent 32-wide stationary
    tiles, one per K-subset) for ~4x PE throughput at batch=32.
  - The 4 column-group partials land on partition groups 32j..32j+32 of
    PSUM; a second small PE pass multiplies by a selector matrix
    S[p,b] = (p%32==b) which fuses the 4-way reduction with the
    (batch,node) -> (node,batch) transpose.
  - Bias + Michaelis-Menten activation on DVE, then the (1024,32) bf16
    chunk is AllGathered across the 8 cores for the next step.
"""

import os
import sys
import types

sys.path.insert(0, "/opt/trn_rl_repo")

import numpy as np
import ml_dtypes

import concourse.bass as bass
import concourse.mybir as mybir
import concourse.tile as tile
from concourse import bacc
import concourse.bass_utils as bass_utils
from concourse.bass import ts
from concourse.bass_utils import run_bass_kernel_spmd

N_NODES = 8192
N_CORES = 8
BATCH = 32
MAX_STEPS = 50
LEAK = 0.01
LOCAL = N_NODES // N_CORES          # 1024 rows per core
K_TILES = N_NODES // 128            # 64
LOCAL_TILES = LOCAL // 128          # 8
CHUNK_F = LOCAL_TILES * BATCH       # 256 free elems per activated chunk

LAST_RESULTS = None  # BassKernelResults of the most recent run (for test.py)


def setup_tracing():
    """Register the axon NTFF profile hook; the container's antenv is a stub."""
    try:
        import antenv
        if "antenv.axon_hooks" not in sys.modules:
            mod = types.ModuleType("antenv.axon_hooks")
            mod._hook = None
            mod.set_axon_ntff_profile_hook = lambda h: setattr(mod, "_hook", h)
            mod.get_axon_ntff_profile_hook = lambda: mod._hook
            sys.modules["antenv.axon_hooks"] = mod
            antenv.axon_hooks = mod
            from trn_agent_boot.trn_boot import _ntff_profile_via_ctypes
            mod.set_axon_ntff_profile_hook(
                _ntff_profile_via_ctypes("/opt/axon/libaxon_pjrt.so")
            )
        bass_utils.upload_artifacts = lambda tmpdir: f"local://{tmpdir}"
    except Exception:
        pass


def build_nc():
    nc = bacc.Bacc(None, target_bir_lowering=False, num_devices=N_CORES)
    f32 = mybir.dt.float32
    bf16 = mybir.dt.bfloat16

    # Per-core inputs (shapes identical on every core; contents sharded).
    wt = nc.dram_tensor("wt", [N_NODES, LOCAL], bf16, kind="ExternalInput")
    xb = nc.dram_tensor("xb", [128, CHUNK_F], f32, kind="ExternalInput")
    s_in = nc.dram_tensor("s_in", [128, BATCH], bf16, kind="ExternalInput")
    out = nc.dram_tensor("out", [128, CHUNK_F], f32, kind="ExternalOutput")

    with tile.TileContext(nc) as tc:
        with (
            tc.tile_pool(name="persist", bufs=1) as persist,
            tc.tile_pool(name="ys", bufs=2) as ys_pool,
            tc.tile_pool(name="chain", bufs=2) as chain,
            tc.tile_pool(name="stage", bufs=2) as stage_pool,
            tc.tile_pool(name="psum", bufs=2, space="PSUM") as psum_pool,
            tc.tile_pool(name="psumt", bufs=2, space="PSUM") as psumt_pool,
            tc.tile_pool(name="dram", bufs=2, space="DRAM") as dram,
        ):
            # ---- persistent SBUF tensors -------------------------------
            wt_sb = persist.tile([128, K_TILES, LOCAL], bf16)      # 128 KB/part
            nc.sync.dma_start(
                out=wt_sb, in_=wt.rearrange("(t p) n -> p t n", p=128)
            )
            xb_sb = persist.tile([128, CHUNK_F], f32)
            nc.sync.dma_start(out=xb_sb, in_=xb[:])
            s_sb = persist.tile([128, BATCH], bf16)
            nc.sync.dma_start(out=s_sb, in_=s_in[:])
            x_sb = persist.tile([128, K_TILES * BATCH], bf16)      # gathered state

            def activation(z_src, to_bf, also_f32=None, width=CHUNK_F):
                """to_bf[:] = mml(z_src) in bf16; optionally also f32 copy.

                mml(z) = max(leak*z, min(z, 1 - 0.25/max(z, 0.5)))
                (exact for |z| < ~99, which holds here).
                """
                m_t = chain.tile([128, width], f32, tag="m", name="m_t")
                nc.vector.tensor_scalar_max(m_t, z_src, 0.5)
                r_t = chain.tile([128, width], f32, tag="r", name="r_t")
                nc.vector.reciprocal_approx_fast(out=r_t, in_=m_t)
                s_t = chain.tile([128, width], f32, tag="s", name="s_t")
                nc.vector.tensor_scalar(
                    s_t, r_t, -0.25, 1.0,
                    mybir.AluOpType.mult, mybir.AluOpType.add,
                )
                t_t = chain.tile([128, width], f32, tag="t", name="t_t")
                nc.vector.tensor_tensor(t_t, z_src, s_t, mybir.AluOpType.min)
                # out = (z * leak) max t
                nc.vector.scalar_tensor_tensor(
                    to_bf, z_src, LEAK, t_t,
                    mybir.AluOpType.mult, mybir.AluOpType.max,
                )
                if also_f32 is not None:
                    nc.vector.scalar_tensor_tensor(
                        also_f32, z_src, LEAK, t_t,
                        mybir.AluOpType.mult, mybir.AluOpType.max,
                    )

            def broadcast(stage_bf):
                """AllGather the local activated chunk into x_sb."""
                ag_in = dram.tile([128, CHUNK_F], bf16, tag="agi")
                nc.sync.dma_start(out=ag_in, in_=stage_bf)
                ag_out = dram.tile(
                    [128 * N_CORES, CHUNK_F], bf16, addr_space="Shared", tag="ago"
                )
                nc.gpsimd.collective_compute(
                    "AllGather",
                    mybir.AluOpType.bypass,
                    replica_groups=[list(range(N_CORES))],
                    ins=[ag_in.opt()],
                    outs=[ag_out.opt()],
                )
                # two DMAs so their fixed latencies overlap
                x_v = x_sb.rearrange("p (c f) -> p c f", c=N_CORES)
                ag_v = ag_out.rearrange("(c p) f -> p c f", p=128)
                nc.sync.dma_start(out=x_v[:, 0:4, :], in_=ag_v[:, 0:4, :])
                nc.sync.dma_start(out=x_v[:, 4:8, :], in_=ag_v[:, 4:8, :])

            # ---- step 1: X1 = mml(X_bias) ------------------------------
            stage_bf = stage_pool.tile([128, CHUNK_F], bf16, tag="stage")
            activation(xb_sb, stage_bf)
            broadcast(stage_bf)

            # ---- steps 2..50: X <- mml(W @ X + X_bias) -----------------
            n_quads = K_TILES // 4  # 16
            for step in range(MAX_STEPS - 1):
                last = step == MAX_STEPS - 2
                # main matmul: 4-way column-tiled over K-subsets
                psum_h = [
                    psum_pool.tile(
                        [128, 512], mybir.dt.float32, tag="pa", name="psum_a"
                    ),
                    psum_pool.tile(
                        [128, 512], mybir.dt.float32, tag="pb", name="psum_b"
                    ),
                ]
                for q in range(n_quads):
                    for h in range(2):
                        for j in range(4):
                            k = 4 * q + j
                            nc.tensor.matmul(
                                psum_h[h][32 * j : 32 * (j + 1), :],
                                x_sb[:, ts(k, BATCH)],
                                wt_sb[:, k, ts(h, 512)],
                                start=(q == 0),
                                stop=(q == n_quads - 1),
                                tile_position=(0, 32 * j),
                            )
                # reduce(4 groups) + transpose via S-matrix PE pass, then
                # bias + activation — processed in two halves so the DVE
                # chain of half 0 overlaps the S-pass of half 1.
                stage_bf = stage_pool.tile([128, CHUNK_F], bf16, tag="stage")
                out_f32 = None
                if last:
                    out_f32 = stage_pool.tile(
                        [128, CHUNK_F], mybir.dt.float32, tag="of", name="out_f32"
                    )
                psum_t = psumt_pool.tile([128, CHUNK_F], mybir.dt.float32, tag="pt")
                for half in range(2):
                    ysb = ys_pool.tile([128, 512], bf16, tag="ysb", name="ysb")
                    nc.vector.tensor_copy(ysb, psum_h[half])
                    for tt_ in range(4):
                        t = 4 * half + tt_
                        nc.tensor.matmul(
                            psum_t[:, ts(t, BATCH)],
                            ysb[:, ts(tt_, 128)],
                            s_sb,
                            start=True,
                            stop=True,
                        )
                    hs = ts(half, 128)
                    z_t = chain.tile([128, 128], mybir.dt.float32, tag="z", name="z_t")
                    nc.vector.tensor_tensor(
                        z_t, psum_t[:, hs], xb_sb[:, hs], mybir.AluOpType.add
                    )
                    activation(
                        z_t,
                        stage_bf[:, hs],
                        also_f32=None if out_f32 is None else out_f32[:, hs],
                        width=128,
                    )
                if last:
                    nc.sync.dma_start(out=out[:], in_=out_f32)
                else:
                    broadcast(stage_bf)

    nc.compile()
    return nc


def _prepare_in_maps(X_full, weights, bias, edge_mask):
    W = np.where(edge_mask, weights, 0.0).astype(np.float32)
    Xb = X_full.astype(np.float32).T + bias.astype(np.float32)  # (n, B)
    S = np.zeros((128, BATCH), np.float32)
    S[np.arange(128), np.arange(128) % BATCH] = 1.0
    S = S.astype(ml_dtypes.bfloat16)
    in_maps = []
    for c in range(N_CORES):
        rows = slice(LOCAL * c, LOCAL * (c + 1))
        wt_c = np.ascontiguousarray(W[rows, :].T).astype(ml_dtypes.bfloat16)
        xb_c = (
            Xb[rows]                       # (1024, 32)
            .reshape(LOCAL_TILES, 128, BATCH)
            .transpose(1, 0, 2)
            .reshape(128, CHUNK_F)
            .copy()
        )
        in_maps.append({"wt": wt_c, "xb": xb_c, "s_in": S})
    return in_maps


def _reassemble(results):
    out = np.empty((BATCH, N_NODES), np.float32)
    for c in range(N_CORES):
        oc = np.asarray(results[c]["out"])  # (128, 256)
        chunk = (
            oc.reshape(128, LOCAL_TILES, BATCH)
            .transpose(1, 0, 2)
            .reshape(LOCAL, BATCH)
        )
        out[:, LOCAL * c : LOCAL * (c + 1)] = chunk.T
    return out


def kernel(X_full, weights, bias, edge_mask):
    global LAST_RESULTS
    setup_tracing()
    in_maps = _prepare_in_maps(X_full, weights, bias, edge_mask)
    nc = build_nc()
    res = run_bass_kernel_spmd(nc, in_maps, core_ids=list(range(N_CORES)))
    LAST_RESULTS = res
    return _reassemble(res.results)


if __name__ == "__main__":
    # quick self-run with random data
    rng = np.random.default_rng(0)
    X_full = rng.random((BATCH, N_NODES), np.float32)
    weights = rng.standard_normal((N_NODES, N_NODES), np.float32)
    bias = 0.001 * np.ones((N_NODES, 1), np.float32)
    edge_mask = rng.random((N_NODES, N_NODES)) < 0.002
    out = kernel(X_full, weights, bias, edge_mask)
    print("out", out.shape, out.dtype, out[:2, :4])
